# revision 1
# baseline (speedup 1.0000x reference)
"""DenseAtt pairwise-MLP attention kernel for 8x Trainium2 NeuronCores.

Reference computation (N=1024, D=64, WIDTH=64, HEADS=4, ALPHA=0.2):
    hi = x @ W1a.T ; hj = x @ W1b.T
    h  = lrelu(hi[:,None,:] + hj[None,:,:] + b1)     # [n, n, 64]
    h  = lrelu(h @ W2.T + b2)                        # [n, n, 64]
    s  = lrelu(h @ W3.T + b3)                        # [n, n, 4]
    attn = softmax(s.reshape(4, n, n), axis=-1)      # C-order reshape quirk

The C-order reshape makes each softmax row of attn.reshape(4096, 1024) equal a
contiguous 1024-element chunk of s.flatten():  row R=(i//256)*1024+(i%256)*4+r
covers s[i, 256r:256(r+1), :] with c = jj*4 + h.  Rows [512k, 512k+512) depend
only on i in [128k, 128k+128)  ->  shard i across 8 cores, no collectives.

Per-core dataflow (width kept in partitions, "transposed" layout):
  - hjT [64,1024] block-packed to [128,512]: rows 0:64 = j in [0,512),
    rows 64:128 = j in [512,1024). ciT[:,i] = W1a@x_i + b1, duplicated to 128.
  - per i: ACT Lrelu(hjT_p + ci bias) -> r1;  blockdiag-W2 matmul -> PSUM;
    ACT Lrelu(+b2) -> r2; blockdiag-W3 matmul (M=32, col tile_position 32*(i%4))
    packs 4 i's into one PSUM bank [128,512].
  - per 4 i's: ACT Lrelu(+b3 pattern), ACT Exp, DVE segment-reduce [128,2].
  - per 32 i's: ones-matmul sums over heads -> Z[8,16], DVE reciprocal,
    ones-matmul broadcast back -> [128,16]; DVE multiply normalizes; DVE 32x32
    stream-transpose puts heads into the free dim; one strided DMA per 32 i's
    writes the (heads,n,n)-interleaved rows (16B inner runs).
"""

import numpy as np
from contextlib import ExitStack

N, D, HEADS, WIDTH, ALPHA = 1024, 64, 4, 64, 0.2
NCORES = 8
IPC = N // NCORES  # 128 i-rows per core

_PROG_CACHE = {}


def _build_program(n_i=IPC, use_lrelu=True, act_dt_name="float32", nrep=1):
    import sys
    if '/opt/trn_rl_repo' not in sys.path:
        sys.path.insert(0, '/opt/trn_rl_repo')
    import concourse.bass as bass
    import concourse.bacc as bacc
    import concourse.tile as tile
    from concourse import mybir

    f32 = mybir.dt.float32
    act_dt = getattr(mybir.dt, act_dt_name)
    AF = mybir.ActivationFunctionType
    func = AF.Prelu if use_lrelu else AF.Relu
    alpha = ALPHA if use_lrelu else 0.0
    NG = n_i // 32          # 32-i groups
    assert n_i % 32 == 0

    nc = bacc.Bacc("TRN2", target_bir_lowering=False)
    CTOT = N + n_i + 64 + 64 + 1 + 128 + 32 + 1 + 1 + 8 + 128
    cb_d = nc.declare_dram_parameter("cb", [128, CTOT], f32, isOutput=False)
    cbh_d = nc.declare_dram_parameter("cbh", [128, 160], act_dt, isOutput=False)
    out_d  = nc.declare_dram_parameter("out",  [4 * n_i, N], f32, isOutput=True)

    with ExitStack() as ctx:
        tc = ctx.enter_context(tile.TileContext(nc))
        cp = ctx.enter_context(tc.tile_pool(name="consts", bufs=1))
        rp = ctx.enter_context(tc.tile_pool(name="work", bufs=3))
        gp = ctx.enter_context(tc.tile_pool(name="groups", bufs=2))
        pw2 = ctx.enter_context(tc.tile_pool(name="pw2", bufs=3, space="PSUM"))
        ps4 = ctx.enter_context(tc.tile_pool(name="ps4", bufs=2, space="PSUM"))
        ptiny = ctx.enter_context(tc.tile_pool(name="ptiny", bufs=1, space="PSUM"))

        # ---- load all constants with ONE DMA (single wait lane for PE) ----
        cb = cp.tile([128, CTOT], f32)
        nc.sync.dma_start(cb[:], cb_d[:])
        cbh = cp.tile([128, 160], act_dt)
        nc.sync.dma_start(cbh[:], cbh_d[:])
        W2bdh = cbh[:, 0:128]
        W3bdh = cbh[:, 128:160]
        o = 0
        xT = cb[0:64, o:o + N]; o += N
        xTi = cb[0:64, o:o + n_i]; o += n_i
        W1aT = cb[0:64, o:o + 64]; o += 64
        W1bT = cb[0:64, o:o + 64]; o += 64
        b1p = cb[:, o:o + 1]; o += 1
        W2bd = cb[:, o:o + 128]; o += 128
        W3bd = cb[:, o:o + 32]; o += 32
        b2p = cb[:, o:o + 1]; o += 1
        b3pat = cb[:, o:o + 1]; o += 1
        osum = cb[:, o:o + 8]; o += 8
        obc = cb[0:8, o:o + 128]; o += 128

        # ---- precompute hjT packed + ciT packed ----
        hj_ps = pw2.tile([128, 512], f32, tag="pw2")
        nc.tensor.matmul(hj_ps[0:64, :], W1bT, xT[:, 0:512],
                         start=True, stop=True)
        nc.tensor.matmul(hj_ps[64:128, :], W1bT, xT[:, 512:1024],
                         start=True, stop=True, tile_position=(0, 64))
        hjT_p = cp.tile([128, 512], act_dt)
        nc.scalar.copy(hjT_p[:], hj_ps[:])

        ci_ps = pw2.tile([128, 512], f32, tag="pw2")
        nc.tensor.matmul(ci_ps[0:64, 0:n_i], W1aT, xTi,
                         start=True, stop=True)
        nc.tensor.matmul(ci_ps[64:128, 0:n_i], W1aT, xTi,
                         start=True, stop=True, tile_position=(0, 64))
        ciT_p = cp.tile([128, n_i], f32)
        nc.vector.tensor_scalar(ciT_p[:], ci_ps[:, 0:n_i], b1p, None,
                                op0=bass.mybir.AluOpType.add)

        # ---- main loop (nrep>1 only for timing benches) ----
        for rep in range(nrep):
          for G in range(NG):
              E32 = gp.tile([128, 4096], f32, tag="E32")
              VT32 = gp.tile([128, 4096], f32, tag="VT32")
              P32 = gp.tile([128, 16], f32, tag="P32")
              for g in range(8):
                  s4_ps = ps4.tile([128, 512], f32, tag="ps4")
                  for i4 in range(4):
                      ip = 32 * G + 4 * g + i4
                      r1 = rp.tile([128, 512], act_dt, tag="r1")
                      nc.scalar.activation(r1[:], hjT_p[:], func,
                                           bias=ciT_p[:, ip:ip + 1], scale=1.0,
                                           alpha=alpha)
                      w2_ps = pw2.tile([128, 512], f32, tag="pw2")
                      nc.tensor.matmul(w2_ps[:], W2bdh, r1[:],
                                       start=True, stop=True)
                      r2 = rp.tile([128, 512], act_dt, tag="r2")
                      nc.scalar.activation(r2[:], w2_ps[:], func,
                                           bias=b2p, scale=1.0, alpha=alpha)
                      nc.tensor.matmul(s4_ps[32 * i4:32 * i4 + 32, :],
                                       W3bdh, r2[:], start=True, stop=True,
                                       tile_position=(0, 32 * i4))
                  el = rp.tile([128, 512], f32, tag="el")
                  nc.scalar.activation(el[:], s4_ps[:], func,
                                       bias=b3pat, scale=1.0, alpha=alpha)
                  # E32 col layout (per g-slice): col = 256*q + 32*jjY + jjX32
                  # with j-within-half = 256*q + 8*jjX32 + jjY.  After the 32x32
                  # stream transpose the HBM side becomes fully contiguous per
                  # (g, f, q): one 16KB DMA each.
                  eslc = E32[:, 512 * g:512 * (g + 1)]
                  escr = eslc.rearrange("p (q Y X) -> p q X Y", q=2, Y=8, X=32)
                  eplain = el[:].rearrange("p (q X Y) -> p q X Y", q=2, X=32, Y=8)
                  nc.scalar.activation(escr, eplain, AF.Exp)
                  nc.vector.tensor_reduce(
                      P32[:, 2 * g:2 * g + 2],
                      eslc.rearrange("p (q j) -> p q j", q=2),
                      axis=mybir.AxisListType.X,
                      op=bass.mybir.AluOpType.add)
              z_ps = ptiny.tile([8, 16], f32, tag="z")
              nc.tensor.matmul(z_ps[:], osum, P32[:], start=True, stop=True)
              zr = rp.tile([8, 16], f32, tag="zr")
              nc.vector.reciprocal(zr[:], z_ps[:])
              rb_ps = ptiny.tile([128, 16], f32, tag="rb")
              nc.tensor.matmul(rb_ps[:], obc, zr[:], start=True, stop=True)
              for g in range(8):
                  eslc = E32[:, 512 * g:512 * (g + 1)]
                  fslc = VT32[:, 512 * g:512 * (g + 1)]
                  F4 = rp.tile([128, 512], f32, tag="F4")
                  rb = rb_ps[:, 2 * g:2 * g + 2].unsqueeze(2)
                  nc.vector.tensor_tensor(
                      F4[:].rearrange("p (q j) -> p q j", q=2),
                      eslc.rearrange("p (q j) -> p q j", q=2),
                      rb.broadcast_to((128, 2, 256)),
                      op=bass.mybir.AluOpType.mult)
                  nc.vector.transpose(fslc, F4[:])
              # After transpose: VT32[32*i4 + jjX32, 512*g + 256*q + 32*jjY + 4*f + h]
              # HBM row = 4*(32*G + 4*g + i4) + 2*f + q; col = 32*jjX32 + 4*jjY + h
              hbm6 = out_d.rearrange("(A f q) (X Y hh) -> A f q X Y hh",
                                     f=2, q=2, X=32, Y=8, hh=4)
              for g in range(8):
                  for f in range(2):
                      for q in range(2):
                          src = VT32[:, 512 * g + 256 * q:512 * g + 256 * (q + 1)].rearrange(
                              "p (Y s) -> p Y s", Y=8, s=32)[:, :, 4 * f:4 * f + 4]
                          A0 = 32 * G + 4 * g
                          nc.sync.dma_start(hbm6[A0:A0 + 4, f, q], src)
    nc.compile()
    return nc


def _host_inputs(x, W1, b1, W2, b2, W3, b3, core, n_i=IPC, act_dt=None):
    import ml_dtypes
    if act_dt is None:
        act_dt = ml_dtypes.bfloat16
    W1a, W1b = W1[:, :D], W1[:, D:]
    xT = np.ascontiguousarray(x.T).astype(np.float32)
    i0 = core * n_i
    W2bd = np.zeros((128, 128), np.float32)
    W2bd[:64, :64] = W2.T
    W2bd[64:, 64:] = W2.T
    W3bd = np.zeros((128, 32), np.float32)
    W3bd[:64, :4] = W3.T
    W3bd[64:, 4:8] = W3.T
    b3pat = np.zeros((128, 1), np.float32)
    for q in range(4):
        for c in range(8):
            b3pat[32 * q + c, 0] = b3[c % 4]
    osum = np.zeros((128, 8), np.float32)
    obc = np.zeros((8, 128), np.float32)
    for i4 in range(4):
        for half in range(2):
            m = 2 * i4 + half
            for hh in range(4):
                p = 32 * i4 + 4 * half + hh
                osum[p, m] = 1.0
                obc[m, p] = 1.0
    CTOT = 1024 + n_i + 64 + 64 + 1 + 128 + 32 + 1 + 1 + 8 + 128
    cb = np.zeros((128, CTOT), np.float32)
    o = 0
    cb[0:64, o:o + 1024] = xT; o += 1024
    cb[0:64, o:o + n_i] = xT[:, i0:i0 + n_i]; o += n_i
    cb[0:64, o:o + 64] = W1a.T; o += 64
    cb[0:64, o:o + 64] = W1b.T; o += 64
    cb[:, o] = np.concatenate([b1, b1]); o += 1
    cb[:, o:o + 128] = W2bd; o += 128
    cb[:, o:o + 32] = W3bd; o += 32
    cb[:, o] = np.concatenate([b2, b2]); o += 1
    cb[:, o] = b3pat[:, 0]; o += 1
    cb[:, o:o + 8] = osum; o += 8
    cb[0:8, o:o + 128] = obc; o += 128
    cbh = np.zeros((128, 160), np.float32)
    cbh[:, 0:128] = W2bd
    cbh[:, 128:160] = W3bd
    return {"cb": cb, "cbh": cbh.astype(act_dt)}


def kernel(x, W1, b1, W2, b2, W3, b3):
    import sys
    if '/opt/trn_rl_repo' not in sys.path:
        sys.path.insert(0, '/opt/trn_rl_repo')
    from concourse.bass_utils import run_bass_kernel_spmd

    key = (IPC, True, "bfloat16")
    if key not in _PROG_CACHE:
        _PROG_CACHE[key] = _build_program(*key)
    nc = _PROG_CACHE[key]

    x = np.asarray(x, np.float32)
    in_maps = [
        _host_inputs(x, np.asarray(W1, np.float32), np.asarray(b1, np.float32),
                     np.asarray(W2, np.float32), np.asarray(b2, np.float32),
                     np.asarray(W3, np.float32), np.asarray(b3, np.float32), k)
        for k in range(NCORES)
    ]
    res = run_bass_kernel_spmd(nc, in_maps, list(range(NCORES)))
    rows = np.concatenate([res.results[k]["out"] for k in range(NCORES)], axis=0)
    return rows.reshape(HEADS, N, N)



# revision 14
# speedup vs baseline: 3.8407x; 3.8407x over previous
"""DenseAtt pairwise-MLP attention kernel for 8x Trainium2 NeuronCores.

Reference computation (N=1024, D=64, WIDTH=64, HEADS=4, ALPHA=0.2):
    hi = x @ W1a.T ; hj = x @ W1b.T
    h  = lrelu(hi[:,None,:] + hj[None,:,:] + b1)     # [n, n, 64]
    h  = lrelu(h @ W2.T + b2)                        # [n, n, 64]
    s  = lrelu(h @ W3.T + b3)                        # [n, n, 4]
    attn = softmax(s.reshape(4, n, n), axis=-1)      # C-order reshape quirk

The C-order reshape means output row R = 4*i + Q (Q = j>>8) has col
c = 4*(j%256) + h, softmax-normalized over the 1024 cols.  Rows
[512k, 512k+512) depend only on i in [128k, 128k+128) -> shard i across
8 cores, no collectives.

Per-core dataflow (width in partitions, j in free):
  - hjT [64,1024] block-packed to [128,512] (rows 0:64 j<512, 64:128
    j>=512).  ciT[:,i] = W1a@x_i + b1 duplicated to both halves.
  - per i: ACT lrelu(hjT + ci bias) -> r1 [128,512] bf16; blockdiag-W2
    matmul -> w2b PSUM slice; per 4 i: one ACT lrelu(+b2) [128,2048]
    -> r2b; blockdiag-W3 matmuls (M=32, col tile_position 32*i4) pack
    4 i into s4b PSUM [128 rows = 32*i4 + 4*fhat + h, 512 = 256q+jw].
  - per 2 g (8 i): ACT lrelu(+b3) [128,1024] -> EL big tile [128,4096]
    per 32-i group; free = 512*g + 256*q + jw.
  - T1 (DVE 32x32 stream transpose, plain [128,4096]):
      TA[32*i4 + jl][512g + 256q + 32jh + (4f+h)] = EL[32*i4 + 4f+h][...jw=32jh+jl]
  - T2 (DVE transpose on strided views) pulls (g,q,f) into partitions:
      OUT[32*i4 + 4g+2q+f][128jh + 4jl + h]  ==  row (i, Q) dense, cols
      in final order 4*jw + h.
  - exp ACT with accum_out -> row sums Z free; reciprocal; per-partition
    scale -> OUTN; one DMA per 32-i group writes 128 x 4KB contiguous
    HBM rows.
"""

import numpy as np
from contextlib import ExitStack

N, D, HEADS, WIDTH, ALPHA = 1024, 64, 4, 64, 0.2
NCORES = 8
IPC = N // NCORES  # 128 i-rows per core

_PROG_CACHE = {}

CB_COLS = 1024 + IPC + 64 + 64 + 1 + 1 + 1  # xT xTi W1aT W1bT b1p b2p b3pat


def _build_program(n_i=IPC, act_dt_name="bfloat16"):
    import sys
    if '/opt/trn_rl_repo' not in sys.path:
        sys.path.insert(0, '/opt/trn_rl_repo')
    import concourse.bass as bass
    import concourse.bacc as bacc
    import concourse.tile as tile
    from concourse import mybir

    f32 = mybir.dt.float32
    act_dt = getattr(mybir.dt, act_dt_name)
    AF = mybir.ActivationFunctionType
    NG = n_i // 32          # 32-i groups
    assert n_i % 32 == 0

    nc = bacc.Bacc("TRN2", target_bir_lowering=False)
    cb_d = nc.declare_dram_parameter("cb", [128, CB_COLS], f32, isOutput=False)
    cbh_d = nc.declare_dram_parameter("cbh", [128, 160], act_dt, isOutput=False)
    out_d = nc.declare_dram_parameter("out", [4 * n_i, N], f32, isOutput=True)

    with ExitStack() as ctx:
        tc = ctx.enter_context(tile.TileContext(nc))
        cp = ctx.enter_context(tc.tile_pool(name="consts", bufs=1))
        rp = ctx.enter_context(tc.tile_pool(name="work", bufs=3))
        gp = ctx.enter_context(tc.tile_pool(name="groups", bufs=2))
        pw = ctx.enter_context(tc.tile_pool(name="pw", bufs=1, space="PSUM"))
        ps4 = ctx.enter_context(tc.tile_pool(name="ps4", bufs=2, space="PSUM"))

        # ---- load all constants with ONE DMA each ----
        cb = cp.tile([128, CB_COLS], f32)
        nc.sync.dma_start(cb[:], cb_d[:])
        cbh = cp.tile([128, 160], act_dt)
        nc.sync.dma_start(cbh[:], cbh_d[:])
        W2bdh = cbh[:, 0:128]
        W3bdh = cbh[:, 128:160]
        o = 0
        xT = cb[0:64, o:o + N]; o += N
        xTi = cb[0:64, o:o + n_i]; o += n_i
        W1aT = cb[0:64, o:o + 64]; o += 64
        W1bT = cb[0:64, o:o + 64]; o += 64
        b1p = cb[:, o:o + 1]; o += 1
        b2p = cb[:, o:o + 1]; o += 1
        b3pat = cb[:, o:o + 1]; o += 1

        # ---- precompute hjT packed + ciT (reuses main-loop PSUM bufs) ----
        hj_ps = pw.tile([128, 2048], f32, tag="w2b")
        nc.tensor.matmul(hj_ps[0:64, 0:512], W1bT, xT[:, 0:512],
                         start=True, stop=True)
        nc.tensor.matmul(hj_ps[64:128, 0:512], W1bT, xT[:, 512:1024],
                         start=True, stop=True, tile_position=(0, 64))
        hjT_p = cp.tile([128, 512], act_dt)
        nc.scalar.copy(hjT_p[:], hj_ps[:, 0:512])

        ci_ps = ps4.tile([128, 1024], f32, tag="s4b")
        nc.tensor.matmul(ci_ps[0:64, 0:n_i], W1aT, xTi,
                         start=True, stop=True)
        nc.tensor.matmul(ci_ps[64:128, 0:n_i], W1aT, xTi,
                         start=True, stop=True, tile_position=(0, 64))
        ciT_p = cp.tile([128, n_i], f32)
        nc.vector.tensor_scalar(ciT_p[:], ci_ps[:, 0:n_i], b1p, None,
                                op0=bass.mybir.AluOpType.add)

        # HBM row view: row = 128*G + 16*g + 4*i4 + 2*f + q, iterated in
        # partition order p = 32*i4 + 16*f + 2*g + q.
        hbmv = out_d.rearrange("(G gg ii ff qq) c -> G ii ff gg qq c",
                               gg=8, ii=4, ff=2, qq=2)

        # ---- main loop over 32-i groups ----
        for G in range(NG):
            EL = gp.tile([128, 4096], f32, tag="EL")
            for g in range(8):
                w2b = pw.tile([128, 2048], f32, tag="w2b")
                for i4 in range(4):
                    ip = 32 * G + 4 * g + i4
                    r1 = rp.tile([128, 512], act_dt, tag="r1")
                    nc.scalar.activation(r1[:], hjT_p[:], AF.Prelu,
                                         bias=ciT_p[:, ip:ip + 1], scale=1.0,
                                         alpha=ALPHA)
                    nc.tensor.matmul(w2b[:, 512 * i4:512 * (i4 + 1)], W2bdh,
                                     r1[:], start=True, stop=True)
                r2b = rp.tile([128, 2048], act_dt, tag="r2b")
                nc.scalar.activation(r2b[:], w2b[:], AF.Prelu, bias=b2p,
                                     scale=1.0, alpha=ALPHA)
                if g % 2 == 0:
                    s4b = ps4.tile([128, 1024], f32, tag="s4b")
                for i4 in range(4):
                    nc.tensor.matmul(
                        s4b[32 * i4:32 * i4 + 32, 512 * (g % 2):512 * (g % 2 + 1)],
                        W3bdh, r2b[:, 512 * i4:512 * (i4 + 1)],
                        start=True, stop=True, tile_position=(0, 32 * i4))
                if g % 2 == 1:
                    nc.scalar.activation(EL[:, 1024 * (g // 2):1024 * (g // 2 + 1)],
                                         s4b[:], AF.Prelu, bias=b3pat,
                                         scale=1.0, alpha=ALPHA)
            # T1: swap partition-low5 (2h+f, W3 row order) with jl; dst is
            # the bit-field layout  512*jh + 128*sU + 32*h + 16*f + 2*g + q
            TA = gp.tile([128, 4096], f32, tag="TA")
            t1_in = EL[:].rearrange("p (gq jh jl) -> p gq jh jl",
                                    gq=16, jh=8, jl=32)
            t1_out = TA[:].rearrange("p (jh sl gq) -> p gq jh sl",
                                     jh=8, sl=32, gq=16)
            nc.vector.transpose(t1_out, t1_in)
            # T2: pull (f, g, q) into partitions, jl back to free:
            # OUTp[32*i4 + 16f + 2g + q][128*jh + 4*jl + h]
            OUTp = gp.tile([128, 1024], f32, tag="OUTp")
            t2_in = TA[:].rearrange("p (jh sU h f gq) -> p jh h sU (f gq)",
                                    jh=8, sU=4, h=4, f=2, gq=16)[:, :, :, 0]
            t2_out = OUTp[:].rearrange("p (jh jl h) -> p jh h jl",
                                       jh=8, jl=32, h=4)
            nc.vector.transpose(t2_out, t2_in)
            # exp + row sums (free accumulate), normalize
            EX = gp.tile([128, 1024], f32, tag="EX")
            Z = gp.tile([128, 1], f32, tag="Z")
            nc.scalar.activation(EX[:], OUTp[:], AF.Exp, accum_out=Z[:])
            rz = gp.tile([128, 1], f32, tag="rz")
            nc.vector.reciprocal(rz[:], Z[:])
            OUTN = gp.tile([128, 1024], f32, tag="OUTN")
            nc.vector.tensor_scalar(OUTN[:], EX[:], rz[:], None,
                                    op0=bass.mybir.AluOpType.mult)
            for ii in range(4):
                for ff in range(2):
                    p0 = 32 * ii + 16 * ff
                    nc.sync.dma_start(hbmv[G, ii, ff], OUTN[p0:p0 + 16, :])
    nc.compile()
    return nc


def _host_inputs(x, W1, b1, W2, b2, W3, b3, core, n_i=IPC, act_dt=None):
    import ml_dtypes
    if act_dt is None:
        act_dt = ml_dtypes.bfloat16
    W1a, W1b = W1[:, :D], W1[:, D:]
    xT = np.ascontiguousarray(x.T).astype(np.float32)
    i0 = core * n_i
    W2bd = np.zeros((128, 128), np.float32)
    W2bd[:64, :64] = W2.T
    W2bd[64:, 64:] = W2.T
    W3bd = np.zeros((128, 32), np.float32)
    W3bd[:64, 0:8:2] = W3.T   # half0 -> rows m = 2h
    W3bd[64:, 1:8:2] = W3.T   # half1 -> rows m = 2h + 1
    b3pat = np.asarray([b3[((p % 32) // 2) % 4] for p in range(128)],
                       np.float32)
    cb = np.zeros((128, CB_COLS), np.float32)
    o = 0
    cb[0:64, o:o + 1024] = xT; o += 1024
    cb[0:64, o:o + n_i] = xT[:, i0:i0 + n_i]; o += n_i
    cb[0:64, o:o + 64] = W1a.T; o += 64
    cb[0:64, o:o + 64] = W1b.T; o += 64
    cb[:, o] = np.concatenate([b1, b1]); o += 1
    cb[:, o] = np.concatenate([b2, b2]); o += 1
    cb[:, o] = b3pat; o += 1
    cbh = np.zeros((128, 160), np.float32)
    cbh[:, 0:128] = W2bd
    cbh[:, 128:160] = W3bd
    return {"cb": cb, "cbh": cbh.astype(act_dt)}


def kernel(x, W1, b1, W2, b2, W3, b3):
    import sys
    if '/opt/trn_rl_repo' not in sys.path:
        sys.path.insert(0, '/opt/trn_rl_repo')
    from concourse.bass_utils import run_bass_kernel_spmd

    key = (IPC, "bfloat16")
    if key not in _PROG_CACHE:
        _PROG_CACHE[key] = _build_program(*key)
    nc = _PROG_CACHE[key]

    x = np.asarray(x, np.float32)
    in_maps = [
        _host_inputs(x, np.asarray(W1, np.float32), np.asarray(b1, np.float32),
                     np.asarray(W2, np.float32), np.asarray(b2, np.float32),
                     np.asarray(W3, np.float32), np.asarray(b3, np.float32), k)
        for k in range(NCORES)
    ]
    res = run_bass_kernel_spmd(nc, in_maps, list(range(NCORES)))
    rows = np.concatenate([res.results[k]["out"] for k in range(NCORES)], axis=0)
    return rows.reshape(HEADS, N, N)
